# revision 19
# baseline (speedup 1.0000x reference)
"""Trainium2 Bass kernel for a biased self-attention block (fp8 DoubleRow).

Reference computation (per sample b):
    hn = GroupNorm32(x) * gamma + beta
    q/k/v = 1x1 conv (C x C matmul) of hn
    s = q^T k / sqrt(C)            [hw_q, hw_k]
    attn = softmax(s) * mask; attn /= sum(attn)   (== exp(s)*m / sum(exp(s)*m))
    out = v @ attn^T; y = x + Wo out + bo

Sharding: 8 cores = 4 samples x 2 query-halves. Each core receives its
sample's full x (spatially rotated so its query half occupies positions
0..2047), computes GroupNorm + K/V for all 4096 positions and Q/attention
output for its 2048 queries only. Keys are mask-compacted on the host so
only masked-in columns take part in attention.

All large matmuls run in fp8(e4m3) DoubleRow perf mode (2x PE throughput):
operands are laid out [128, 2, free] where dim1 packs two 128-wide
contraction subtiles (channel-chunk pairs for projections/scores, key-window
pairs for the attention-output accumulation). Scale management keeps every
fp8 operand in the format's normal range:
  - weights are pre-scaled x16 on the host, 1/16 folded downstream;
  - q/k are stored unscaled (~N(0,1)); the 1/sqrt(C) score scale rides the
    Exp activation's scale input, the log-mask (with a -log4 shift for fp8
    headroom) its per-partition bias;
  - the softmax division commutes with the Wo matmul (the denominator is
    constant across channels), so the unnormalized context is cast to fp8
    (/16), Wo is applied, and one fused multiply by 1/(2*sum) lands on the
    output path - the slow reciprocal never touches the PE critical path.

Phase 2 is software-pipelined: scores/exp for window pair wp+1 are issued
ahead of the context-accumulation matmuls of pair wp, so the PE never waits
on the Exp activation. GroupNorm stats use two accum_out passes on DVE
(sum / sum-of-squares) instead of bn_stats, and the DMA lanes are ordered
so the tiny tensors every stats chain needs land before the bulk traffic.
"""

import sys

sys.path.insert(0, "/opt/trn_rl_repo")

import numpy as np
import ml_dtypes

import concourse.bass as bass
import concourse.tile as tile
from concourse import bacc, mybir
from concourse.bass_utils import run_bass_kernel_spmd

F32 = mybir.dt.float32
BF16 = mybir.dt.bfloat16
F8 = mybir.dt.float8e4
AX = mybir.AxisListType
ALU = mybir.AluOpType
ACTF = mybir.ActivationFunctionType
PERF = mybir.MatmulPerfMode.DoubleRow

B, C, HGT, WID = 4, 512, 64, 64
HW = HGT * WID          # 4096
GROUPS = 32
GSIZE = C // GROUPS     # 16 channels per group
EPS = 1e-6
NCH = C // 128          # 4 channel chunks
NCP = NCH // 2          # 2 channel chunk pairs
NQ = HW // 2            # 2048 queries per core
QT = 512                # query tile (matmul free dim)
NQT = NQ // QT          # 4 query tiles
NW = HW // 512          # 8 spatial windows of 512 for projections
NKM = 2304              # compacted (masked-in) key capacity, 18 windows of 128
NKWM = NKM // 128       # 18 key windows after mask compaction
NKWP = NKWM // 2        # 9 key window pairs
NEG = -30000.0          # log(0) stand-in for the additive mask
WSCL = 16.0             # host-side weight scale into fp8
LOGSHIFT = -float(np.log(4.0))  # constant score shift (softmax-invariant)


def build_program(loop_n: int = 1):
    nc = bacc.Bacc()
    x_d = nc.declare_dram_parameter("xbf", [C, HW], BF16, isOutput=False)
    lm_d = nc.declare_dram_parameter("lmask", [NKM], F32, isOutput=False)
    xm_d = nc.declare_dram_parameter("xm", [C, NKM], BF16, isOutput=False)
    gam_d = nc.declare_dram_parameter("gamma", [C], F32, isOutput=False)
    bet_d = nc.declare_dram_parameter("beta", [C], F32, isOutput=False)
    # weights prearranged host-side into the SBUF DoubleRow pair layout:
    # [p, cp*1024 + j*512 + c] = 16*W.T[cp*256 + j*128 + p, c]
    wq_d = nc.declare_dram_parameter("wqt", [128, 4 * C], F8, isOutput=False)
    wk_d = nc.declare_dram_parameter("wkt", [128, 4 * C], F8, isOutput=False)
    wv_d = nc.declare_dram_parameter("wvt", [128, 4 * C], F8, isOutput=False)
    wo_d = nc.declare_dram_parameter("wot", [128, 4 * C], F8, isOutput=False)
    bq_d = nc.declare_dram_parameter("bq2", [C], F32, isOutput=False)
    bk_d = nc.declare_dram_parameter("bk", [C], F32, isOutput=False)
    bo2_d = nc.declare_dram_parameter("bo2", [C], F32, isOutput=False)
    ind_d = nc.declare_dram_parameter("ind", [8, 128], F32, isOutput=False)
    ind2_d = nc.declare_dram_parameter("ind2", [128, 8], F32, isOutput=False)
    y_d = nc.declare_dram_parameter("y", [C, NQ], F32, isOutput=True)

    qscale = 1.0 / np.sqrt(C)

    with tile.TileContext(nc) as tc:
        with tc.tile_pool(name="persist", bufs=1) as pp:
            # --- weights as chunk pairs [128, cp, j, C] for DoubleRow ---
            wq_all = pp.tile([128, NCP, 2, C], F8, tag="wq_all")
            wk_all = pp.tile([128, NCP, 2, C], F8, tag="wk_all")
            wv_all = pp.tile([128, NCP, 2, C], F8, tag="wv_all")
            wo_all = pp.tile([128, NCP, 2, C], F8, tag="wo_all")
            wqp = [wq_all[:, i] for i in range(NCP)]
            wkp = [wk_all[:, i] for i in range(NCP)]
            wvp = [wv_all[:, i] for i in range(NCP)]
            wop = [wo_all[:, i] for i in range(NCP)]

            # per-chunk vectors packed as [128, NCH] (column = chunk)
            gam_sb = pp.tile([128, NCH], F32, tag="gam")
            bet_sb = pp.tile([128, NCH], F32, tag="bet")
            bq_sb = pp.tile([128, NCH], F32, tag="bq2")
            bk_sb = pp.tile([128, NCH], F32, tag="bk")
            bo2_sb = pp.tile([128, NCH], F32, tag="bo2")
            lm_sb = pp.tile([128, NKWM], F32, tag="lmask")
            onesp = pp.tile([128, 2, 128], F8, tag="ones")
            ind_sb = pp.tile([8, 128], F32, tag="ind")
            ind2_sb = pp.tile([128, 8], F32, tag="ind2")

            # --- persistent activations (fp8 pair layouts) ---
            kp = [pp.tile([128, 2, NKM], F8, name="kp", tag=f"kp{i}") for i in range(NCP)]
            qp = [pp.tile([128, 2, NQ], F8, name="qp", tag=f"qp{i}") for i in range(NCP)]
            vtp = [pp.tile([128, 2, C], F8, name="vtp", tag=f"vtp{w}") for w in range(NKWP)]
            # x stays resident (bf16): stats source + residual add
            xf = [pp.tile([128, HW], BF16, name="xf", tag=f"x{i}") for i in range(NCH)]

            # ================= phase 1: groupnorm stats + projections ========
            import contextlib

            loop_cm = tc.For_i(0, loop_n, 1) if loop_n > 1 else contextlib.nullcontext()
            loop_ctx = contextlib.ExitStack()
            loop_ctx.enter_context(loop_cm)
            with (
                tc.tile_pool(name="ph1", bufs=1) as p1,
                tc.tile_pool(name="ph1psum", bufs=1, space="PSUM") as p1p,
            ):
                # DMA lanes: ACT/PE carry only what their engines need before
                # their first compute (tiny chain constants + the early
                # weights); SP and gpsimd carry the bulk, chunk-major so the
                # stats chain for chunk c never waits on chunk c+1 traffic.
                HHW = HW // 2
                XMA = 1152  # xm column split

                def xdma(eng, i, piece):
                    sl = slice(piece * HHW, (piece + 1) * HHW)
                    eng.dma_start(out=xf[i][:, sl], in_=x_d[bass.ts(i, 128), sl])

                def vdma(eng, v_d, t):
                    eng.dma_start(out=t, in_=v_d[:].rearrange("(i p) -> p i", p=128))

                xm_sb = [
                    p1.tile([128, NKM], BF16, name="xm_sb", tag=f"xm{i}")
                    for i in range(NCH)
                ]

                def xmdma(eng, i, piece):
                    sl = slice(0, XMA) if piece == 0 else slice(XMA, NKM)
                    eng.dma_start(out=xm_sb[i][:, sl], in_=xm_d[bass.ts(i, 128), sl])

                # x rides ONLY the two HWDGE queues (SWDGE moves ~30 GB/s and
                # would gate the stats); gpsimd carries just the late-needed
                # V/O weights. Chain constants lead their lanes.
                # ACT lane: chain constants, late x halves, then Q/K weights.
                nc.scalar.dma_start(out=ind2_sb, in_=ind2_d[:, :])
                nc.scalar.dma_start(out=ind_sb, in_=ind_d[:, :])
                vdma(nc.scalar, gam_d, gam_sb)
                vdma(nc.scalar, bet_d, bet_sb)
                xdma(nc.scalar, 2, 1)
                xdma(nc.scalar, 3, 1)
                nc.scalar.dma_start(out=wq_all, in_=wq_d[:, :])
                nc.scalar.dma_start(out=wk_all, in_=wk_d[:, :])
                # SP lane: small vectors, the bulk of x chunk-major, then xm.
                vdma(nc.sync, bq_d, bq_sb)
                vdma(nc.sync, bk_d, bk_sb)
                vdma(nc.sync, bo2_d, bo2_sb)
                nc.sync.dma_start(out=lm_sb, in_=lm_d[:].rearrange("(w p) -> p w", p=128))
                xdma(nc.sync, 0, 0)
                xdma(nc.sync, 0, 1)
                xdma(nc.sync, 1, 0)
                xdma(nc.sync, 1, 1)
                xdma(nc.sync, 2, 0)
                xdma(nc.sync, 3, 0)
                xmdma(nc.sync, 0, 0)
                xmdma(nc.sync, 1, 0)
                xmdma(nc.sync, 2, 0)
                xmdma(nc.sync, 3, 0)
                xmdma(nc.sync, 0, 1)
                xmdma(nc.sync, 1, 1)
                xmdma(nc.sync, 2, 1)
                xmdma(nc.sync, 3, 1)
                # gpsimd (SWDGE) lane: V then O weights only.
                nc.gpsimd.dma_start(out=wv_all, in_=wv_d[:, :])
                nc.gpsimd.dma_start(out=wo_all, in_=wo_d[:, :])
                nc.vector.memset(onesp, 2.0)

                # Per-chunk stats: two accum_out passes on DVE (sum, sumsq)
                # replace bn_stats at ~2.3x the elem rate. Chain ops (tiny)
                # spread over ACT/DVE/PE.
                eps_sb = p1.tile([8, 1], F32, tag="eps")
                nc.vector.memset(eps_sb, EPS)
                scale4 = p1.tile([128, NCH], F32, tag="scale4")
                shift4 = p1.tile([128, NCH], F32, tag="shift4")
                scale_sb = [scale4[:, i : i + 1] for i in range(NCH)]
                shift_sb = [shift4[:, i : i + 1] for i in range(NCH)]
                rnorm = 1.0 / (GSIZE * HW)

                def stats_chunk(i):
                    # sum / sum-of-squares via three full-rate DVE passes
                    # (bn_stats runs at less than half the elem rate)
                    st2 = p1.tile([128, 2], F32, name="st2", tag=f"st2_{i}")
                    trash = p1.tile([128, HW], BF16, name="trash", tag="trash", bufs=2)
                    nc.vector.reduce_sum(out=st2[:, 0:1], in_=xf[i], axis=AX.X)
                    nc.vector.tensor_mul(out=trash, in0=xf[i], in1=xf[i])
                    nc.vector.reduce_sum(out=st2[:, 1:2], in_=trash, axis=AX.X)
                    return st2

                def chain_chunk(i, st2):
                    # st2 cols: sum, sumsq (per channel); group-reduce on PE
                    z_ps = p1p.tile([8, 2], F32, name="z_ps", tag="mr", bufs=2)
                    nc.tensor.matmul(z_ps, ind2_sb, st2, start=True, stop=True)
                    z_sb = p1.tile([8, 2], F32, name="z_sb", tag=f"z_sb{i}")
                    nc.scalar.copy(out=z_sb, in_=z_ps)
                    stat2 = p1.tile([8, 2], F32, name="stat2", tag=f"stat2_{i}")
                    nc.scalar.mul(out=stat2[:, 0:1], in_=z_sb[:, 0:1], mul=rnorm)
                    msq = p1.tile([8, 2], F32, name="msq", tag=f"msq{i}")
                    nc.scalar.activation(out=msq[:, 0:1], in_=stat2[:, 0:1], func=ACTF.Square)
                    # var = sumsq/N - mean^2 ; rstd = 1/sqrt(var+eps)
                    nc.vector.scalar_tensor_tensor(
                        out=msq[:, 1:2], in0=z_sb[:, 1:2], scalar=rnorm,
                        in1=msq[:, 0:1], op0=ALU.mult, op1=ALU.subtract,
                    )
                    nc.scalar.activation(out=msq[:, 1:2], in_=msq[:, 1:2], func=ACTF.Sqrt, bias=eps_sb)
                    nc.vector.reciprocal(out=stat2[:, 1:2], in_=msq[:, 1:2])
                    mr = p1p.tile([128, 2], F32, name="mr", tag="mr", bufs=2)
                    nc.tensor.matmul(mr, ind_sb, stat2, start=True, stop=True)
                    # scale = gamma * rstd ; shift = beta - mean * scale
                    nc.vector.tensor_mul(
                        out=scale_sb[i], in0=gam_sb[:, i : i + 1], in1=mr[:, 1:2]
                    )
                    tmp_sh = p1.tile([128, 1], F32, name="tmp_sh", tag=f"tmp_sh{i}")
                    nc.vector.tensor_scalar_mul(out=tmp_sh, in0=mr[:, 0:1], scalar1=scale_sb[i])
                    nc.vector.tensor_sub(out=shift_sb[i], in0=bet_sb[:, i : i + 1], in1=tmp_sh)

                st2s = {0: stats_chunk(0), 1: stats_chunk(1)}
                chain_chunk(0, st2s[0])
                st2s[2] = stats_chunk(2)
                chain_chunk(1, st2s[1])
                st2s[3] = stats_chunk(3)
                chain_chunk(2, st2s[2])
                chain_chunk(3, st2s[3])

                # projections. q over the local query half (from xf);
                # k/v only over the mask-compacted key columns (xm).
                # hn/hm produced in fp8 pair layout [128, 2, 512] on ACT
                # (per-partition scale+bias activation); q/k bias+1/16 on DVE.
                for nw in range(NW // 2):
                    nsl = bass.ts(nw, 512)
                    hn = []
                    for cp in range(NCP):
                        h_t = p1.tile([128, 2, 512], F8, name="hn", tag="hn", bufs=8)
                        for j in range(2):
                            c = 2 * cp + j
                            nc.scalar.activation(
                                out=h_t[:, j, :], in_=xf[c][:, nsl],
                                func=ACTF.Identity,
                                scale=scale_sb[c], bias=shift_sb[c],
                            )
                        hn.append(h_t)
                    for co in range(NCH):
                        pq = p1p.tile([128, 512], F32, name="pq", tag="pq", bufs=2)
                        for cp in range(NCP):
                            nc.tensor.matmul(
                                pq, wqp[cp][:, :, bass.ts(co, 128)], hn[cp],
                                start=(cp == 0), stop=(cp == NCP - 1),
                                perf_mode=PERF,
                            )
                        nc.vector.tensor_scalar(
                            out=qp[co // 2][:, co % 2, nsl], in0=pq,
                            scalar1=1.0 / WSCL, scalar2=bq_sb[:, co : co + 1],
                            op0=ALU.mult, op1=ALU.add,
                        )
                for mw in range((NKM + 511) // 512):
                    lo = mw * 512
                    wsz = min(512, NKM - lo)
                    msl = slice(lo, lo + wsz)
                    hm = []
                    for cp in range(NCP):
                        h_t = p1.tile([128, 2, 512], F8, name="hm", tag="hn", bufs=8)
                        for j in range(2):
                            c = 2 * cp + j
                            nc.scalar.activation(
                                out=h_t[:, j, :wsz], in_=xm_sb[c][:, msl],
                                func=ACTF.Identity,
                                scale=scale_sb[c], bias=shift_sb[c],
                            )
                        hm.append(h_t)
                    for co in range(NCH):
                        pk = p1p.tile([128, 512], F32, name="pk", tag="pk", bufs=2)
                        for cp in range(NCP):
                            nc.tensor.matmul(
                                pk[:, :wsz], wkp[cp][:, :, bass.ts(co, 128)],
                                hm[cp][:, :, :wsz],
                                start=(cp == 0), stop=(cp == NCP - 1),
                                perf_mode=PERF,
                            )
                        nc.vector.tensor_scalar(
                            out=kp[co // 2][:, co % 2, msl], in0=pk[:, :wsz],
                            scalar1=1.0 / WSCL, scalar2=bk_sb[:, co : co + 1],
                            op0=ALU.mult, op1=ALU.add,
                        )
                    # v, produced transposed and x16: vt[key, c_out] = hm^T @ (16 Wv)
                    for kw in range(wsz // 128):
                        pv = p1p.tile([128, C], F32, name="pv", tag="pv", bufs=2)
                        for cp in range(NCP):
                            nc.tensor.matmul(
                                pv, hm[cp][:, :, bass.ts(kw, 128)], wvp[cp],
                                start=(cp == 0), stop=(cp == NCP - 1),
                                perf_mode=PERF,
                            )
                        kwg = mw * 4 + kw
                        nc.vector.tensor_copy(out=vtp[kwg // 2][:, kwg % 2, :], in_=pv)

            # ================= phase 2: attention =================
            with (
                tc.tile_pool(name="ph2", bufs=1) as p2,
                tc.tile_pool(name="ph2psum", bufs=1, space="PSUM") as p2p,
            ):
                def emit_scores(qt, wp):
                    """scores + exp for window pair wp against query tile qt;
                    returns the fp8 probability pair tile."""
                    qsl = bass.ts(qt, QT)
                    ptp = p2.tile([128, 2, QT], F8, name="ptp", tag="pt", bufs=3)
                    for j in range(2):
                        w = 2 * wp + j
                        sc = p2p.tile([128, QT], F32, name="sc", tag="sc", bufs=3)
                        for cp in range(NCP):
                            nc.tensor.matmul(
                                sc, kp[cp][:, :, bass.ts(w, 128)],
                                qp[cp][:, :, qsl],
                                start=(cp == 0), stop=(cp == NCP - 1),
                                perf_mode=PERF,
                            )
                        # p = exp(s/sqrt(C) + logmask_k - log4)
                        nc.scalar.activation(
                            out=ptp[:, j, :], in_=sc, func=ACTF.Exp,
                            bias=lm_sb[:, w : w + 1], scale=qscale,
                        )
                    return ptp

                items = [(qt, wp) for qt in range(NQT) for wp in range(NKWP)]
                ptp_next = emit_scores(*items[0])
                out_ps = None
                for idx, (qt, wp) in enumerate(items):
                    qsl = bass.ts(qt, QT)
                    ptp_cur = ptp_next
                    # prefetch the next pair's scores so the PE never waits
                    # on this pair's Exp
                    if idx + 1 < len(items):
                        ptp_next = emit_scores(*items[idx + 1])
                    if wp == 0:
                        out_ps = [
                            p2p.tile([128, QT], F32, name="out_ps", tag="out", bufs=4)
                            for _ in range(NCH)
                        ]
                        ds_ps = p2p.tile([128, QT], F32, name="ds_ps", tag="ds", bufs=1)
                    for c in range(NCH):
                        nc.tensor.matmul(
                            out_ps[c], vtp[wp][:, :, bass.ts(c, 128)], ptp_cur,
                            start=(wp == 0), stop=(wp == NKWP - 1),
                            perf_mode=PERF,
                        )
                    nc.tensor.matmul(
                        ds_ps, onesp, ptp_cur,
                        start=(wp == 0), stop=(wp == NKWP - 1),
                        perf_mode=PERF,
                    )
                    if wp != NKWP - 1:
                        continue
                    # ---- query-tile tail ----
                    # out_ps = 16*sum(p v), ds_ps = 2*sum(p). Cast the
                    # unnormalized context to fp8 as sum(p v)/8 (|.| < 90);
                    # the per-query softmax denominator is applied after Wo.
                    onp = [
                        p2.tile([128, 2, QT], F8, name="onp", tag="onp", bufs=2)
                        for _ in range(NCP)
                    ]
                    for c in range(NCH):
                        nc.vector.tensor_scalar_mul(
                            out=onp[c // 2][:, c % 2, :], in0=out_ps[c],
                            scalar1=1.0 / (WSCL * 8.0),
                        )
                    # dinv = 1/(2 sum p); off the PE critical path
                    dinv = p2.tile([128, QT], F32, name="dinv", tag="dinv", bufs=2)
                    nc.vector.reciprocal(out=dinv, in_=ds_ps)
                    for co in range(NCH):
                        pj = p2p.tile([128, QT], F32, name="pj", tag="out", bufs=4)
                        for cp in range(NCP):
                            nc.tensor.matmul(
                                pj, wop[cp][:, :, bass.ts(co, 128)], onp[cp],
                                start=(cp == 0), stop=(cp == NCP - 1),
                                perf_mode=PERF,
                            )
                        # pj = 2*(Wo sum(p v)); pj*dinv = Wo out exactly;
                        # y = pj*dinv + bo2 + x in two DVE ops
                        t2 = p2.tile([128, QT], F32, name="t2", tag="t2", bufs=3)
                        nc.vector.tensor_mul(out=t2, in0=pj, in1=dinv)
                        y_t = p2.tile([128, QT], F32, name="y_t", tag="yt", bufs=3)
                        nc.vector.scalar_tensor_tensor(
                            out=y_t, in0=t2, scalar=bo2_sb[:, co : co + 1],
                            in1=xf[co][:, qsl], op0=ALU.add, op1=ALU.add,
                        )
                        nc.sync.dma_start(out=y_d[bass.ts(co, 128), qsl], in_=y_t)

            loop_ctx.close()

    nc.finalize()
    return nc


_prog_cache = {}


def _get_program(loop_n: int = 1):
    if loop_n not in _prog_cache:
        _prog_cache[loop_n] = build_program(loop_n)
    return _prog_cache[loop_n]


def _to_f8(a):
    return np.clip(a, -240.0, 240.0).astype(ml_dtypes.float8_e4m3)


def _prearrange_w(W):
    # [p, cp*1024 + j*512 + c] = 16*W.T[cp*256 + j*128 + p, c]
    arr = np.ascontiguousarray(np.asarray(W, np.float32).T) * WSCL
    pre = arr.reshape(2, 2, 128, C).transpose(2, 0, 1, 3).reshape(128, 4 * C)
    return _to_f8(pre)


def _prep_in_maps(x, mask, gamma, beta, Wq, bq, Wk, bk, Wv, bv, Wo, bo):
    x = np.asarray(x, np.float32).reshape(B, C, HW)
    mask = np.asarray(mask, np.float32)
    bf = ml_dtypes.bfloat16
    shared = {
        "gamma": np.asarray(gamma, np.float32),
        "beta": np.asarray(beta, np.float32),
        "wqt": _prearrange_w(Wq),
        "wkt": _prearrange_w(Wk),
        "wvt": _prearrange_w(Wv),
        "wot": _prearrange_w(Wo),
        "bq2": np.asarray(bq, np.float32),
        "bk": np.asarray(bk, np.float32),
        "bo2": (np.asarray(Wo, np.float32) @ np.asarray(bv, np.float32)
                + np.asarray(bo, np.float32)).astype(np.float32),
        "ind": (np.arange(128)[None, :] // GSIZE == np.arange(8)[:, None]).astype(
            np.float32
        ),
        "ind2": (np.arange(128)[:, None] // GSIZE == np.arange(8)[None, :]).astype(
            np.float32
        ),
    }
    in_maps = []
    for core in range(8):
        b, half = core // 2, core % 2
        xb, mb = x[b], mask[b]
        if half == 1:
            xb = np.concatenate([xb[:, NQ:], xb[:, :NQ]], axis=1)
            mb = np.concatenate([mb[NQ:], mb[:NQ]])
        # compact the keys: only masked-in columns take part in attention
        idx = np.nonzero(mb > 0.5)[0]
        nk = len(idx)
        assert nk <= NKM, f"mask density too high: {nk} > {NKM}"
        xm = np.zeros((C, NKM), dtype=bf)
        xm[:, :nk] = xb[:, idx].astype(bf)
        lm = np.full(NKM, NEG, np.float32)
        lm[:nk] = LOGSHIFT
        in_maps.append(
            {"xbf": xb.astype(bf), "xm": xm, "lmask": lm, **shared}
        )
    return in_maps


def kernel(x, mask, gamma, beta, Wq, bq, Wk, bk, Wv, bv, Wo, bo):
    nc = _get_program()
    in_maps = _prep_in_maps(x, mask, gamma, beta, Wq, bq, Wk, bk, Wv, bv, Wo, bo)
    res = run_bass_kernel_spmd(nc, in_maps, list(range(8)))
    out = np.empty((B, C, HW), np.float32)
    for core in range(8):
        b, half = core // 2, core % 2
        out[b, :, half * NQ : (half + 1) * NQ] = res.results[core]["y"]
    return out.reshape(B, C, HGT, WID)


# revision 21
# speedup vs baseline: 1.1354x; 1.1354x over previous
"""Trainium2 Bass kernel for a biased self-attention block (fp8 DoubleRow).

Reference computation (per sample b):
    hn = GroupNorm32(x) * gamma + beta
    q/k/v = 1x1 conv (C x C matmul) of hn
    s = q^T k / sqrt(C)            [hw_q, hw_k]
    attn = softmax(s) * mask; attn /= sum(attn)   (== exp(s)*m / sum(exp(s)*m))
    out = v @ attn^T; y = x + Wo out + bo

Sharding: 8 cores = 4 samples x 2 query-halves. Each core receives its
sample's full x (spatially rotated so its query half occupies positions
0..2047), computes GroupNorm + K/V for all 4096 positions and Q/attention
output for its 2048 queries only. Keys are mask-compacted on the host so
only masked-in columns take part in attention.

All large matmuls run in fp8(e4m3) DoubleRow perf mode (2x PE throughput):
operands are laid out [128, 2, free] where dim1 packs two 128-wide
contraction subtiles (channel-chunk pairs for projections/scores, key-window
pairs for the attention-output accumulation). Scale management keeps every
fp8 operand in the format's normal range:
  - weights are pre-scaled x16 on the host, 1/16 folded downstream;
  - q/k are stored unscaled (~N(0,1)); the 1/sqrt(C) score scale rides the
    Exp activation's scale input, the log-mask (with a -log4 shift for fp8
    headroom) its per-partition bias;
  - the softmax division commutes with the Wo matmul (the denominator is
    constant across channels), so the unnormalized context is cast to fp8
    (/16), Wo is applied, and one fused multiply by 1/(2*sum) lands on the
    output path - the slow reciprocal never touches the PE critical path.

Phase 2 is software-pipelined: scores/exp for window pair wp+1 are issued
ahead of the context-accumulation matmuls of pair wp, so the PE never waits
on the Exp activation. GroupNorm stats use two accum_out passes on DVE
(sum / sum-of-squares) instead of bn_stats, and the DMA lanes are ordered
so the tiny tensors every stats chain needs land before the bulk traffic.
"""

import sys

sys.path.insert(0, "/opt/trn_rl_repo")

import numpy as np
import ml_dtypes

import concourse.bass as bass
import concourse.tile as tile
from concourse import bacc, mybir
from concourse.bass_utils import run_bass_kernel_spmd

F32 = mybir.dt.float32
BF16 = mybir.dt.bfloat16
F8 = mybir.dt.float8e4
AX = mybir.AxisListType
ALU = mybir.AluOpType
ACTF = mybir.ActivationFunctionType
PERF = mybir.MatmulPerfMode.DoubleRow

B, C, HGT, WID = 4, 512, 64, 64
HW = HGT * WID          # 4096
GROUPS = 32
GSIZE = C // GROUPS     # 16 channels per group
EPS = 1e-6
NCH = C // 128          # 4 channel chunks
NCP = NCH // 2          # 2 channel chunk pairs
NQ = HW // 2            # 2048 queries per core
QT = 512                # query tile (matmul free dim)
NQT = NQ // QT          # 4 query tiles
NW = HW // 512          # 8 spatial windows of 512 for projections
NKM = 2304              # compacted (masked-in) key capacity, 18 windows of 128
NKWM = NKM // 128       # 18 key windows after mask compaction
NKWP = NKWM // 2        # 9 key window pairs
NEG = -30000.0          # log(0) stand-in for the additive mask
WSCL = 16.0             # host-side weight scale into fp8
LOGSHIFT = -float(np.log(4.0))  # constant score shift (softmax-invariant)


def build_program(loop_n: int = 1):
    nc = bacc.Bacc()
    x_d = nc.declare_dram_parameter("xbf", [C, HW], BF16, isOutput=False)
    lm_d = nc.declare_dram_parameter("lmask", [NKM], F32, isOutput=False)
    xm_d = nc.declare_dram_parameter("xm", [C, NKM], BF16, isOutput=False)
    gam_d = nc.declare_dram_parameter("gamma", [C], F32, isOutput=False)
    bet_d = nc.declare_dram_parameter("beta", [C], F32, isOutput=False)
    # weights prearranged host-side into the SBUF DoubleRow pair layout:
    # [p, cp*1024 + j*512 + c] = 16*W.T[cp*256 + j*128 + p, c]
    wq_d = nc.declare_dram_parameter("wqt", [128, 4 * C], F8, isOutput=False)
    wk_d = nc.declare_dram_parameter("wkt", [128, 4 * C], F8, isOutput=False)
    wv_d = nc.declare_dram_parameter("wvt", [128, 4 * C], F8, isOutput=False)
    wo_d = nc.declare_dram_parameter("wot", [128, 4 * C], F8, isOutput=False)
    bq_d = nc.declare_dram_parameter("bq2", [C], F32, isOutput=False)
    bk_d = nc.declare_dram_parameter("bk", [C], F32, isOutput=False)
    bo2_d = nc.declare_dram_parameter("bo2", [C], F32, isOutput=False)
    ind_d = nc.declare_dram_parameter("ind", [8, 128], F32, isOutput=False)
    ind2_d = nc.declare_dram_parameter("ind2", [128, 8], F32, isOutput=False)
    y_d = nc.declare_dram_parameter("y", [C, NQ], F32, isOutput=True)

    qscale = 1.0 / np.sqrt(C)

    with tile.TileContext(nc) as tc:
        with tc.tile_pool(name="persist", bufs=1) as pp:
            # --- weights as chunk pairs [128, cp, j, C] for DoubleRow ---
            wq_all = pp.tile([128, NCP, 2, C], F8, tag="wq_all")
            wk_all = pp.tile([128, NCP, 2, C], F8, tag="wk_all")
            wv_all = pp.tile([128, NCP, 2, C], F8, tag="wv_all")
            wo_all = pp.tile([128, NCP, 2, C], F8, tag="wo_all")
            wqp = [wq_all[:, i] for i in range(NCP)]
            wkp = [wk_all[:, i] for i in range(NCP)]
            wvp = [wv_all[:, i] for i in range(NCP)]
            wop = [wo_all[:, i] for i in range(NCP)]

            # per-chunk vectors packed as [128, NCH] (column = chunk)
            gam_sb = pp.tile([128, NCH], F32, tag="gam")
            bet_sb = pp.tile([128, NCH], F32, tag="bet")
            bq_sb = pp.tile([128, NCH], F32, tag="bq2")
            bk_sb = pp.tile([128, NCH], F32, tag="bk")
            bo2_sb = pp.tile([128, NCH], F32, tag="bo2")
            lm_sb = pp.tile([128, NKWM], F32, tag="lmask")
            onesp = pp.tile([128, 2, 128], F8, tag="ones")
            ind_sb = pp.tile([8, 128], F32, tag="ind")
            ind2_sb = pp.tile([128, 8], F32, tag="ind2")

            # --- persistent activations (fp8 pair layouts) ---
            kp = [pp.tile([128, 2, NKM], F8, name="kp", tag=f"kp{i}") for i in range(NCP)]
            qp = [pp.tile([128, 2, NQ], F8, name="qp", tag=f"qp{i}") for i in range(NCP)]
            vtp = [pp.tile([128, 2, C], F8, name="vtp", tag=f"vtp{w}") for w in range(NKWP)]
            # x stays resident (bf16): stats source + residual add
            xf = [pp.tile([128, HW], BF16, name="xf", tag=f"x{i}") for i in range(NCH)]

            # ================= phase 1: groupnorm stats + projections ========
            import contextlib

            loop_cm = tc.For_i(0, loop_n, 1) if loop_n > 1 else contextlib.nullcontext()
            loop_ctx = contextlib.ExitStack()
            loop_ctx.enter_context(loop_cm)
            with (
                tc.tile_pool(name="ph1", bufs=1) as p1,
                tc.tile_pool(name="ph1psum", bufs=1, space="PSUM") as p1p,
            ):
                # DMA lanes: ACT/PE carry only what their engines need before
                # their first compute (tiny chain constants + the early
                # weights); SP and gpsimd carry the bulk, chunk-major so the
                # stats chain for chunk c never waits on chunk c+1 traffic.
                HHW = HW // 2
                XMA = 1152  # xm column split

                def xdma(eng, i, piece):
                    sl = slice(piece * HHW, (piece + 1) * HHW)
                    eng.dma_start(out=xf[i][:, sl], in_=x_d[bass.ts(i, 128), sl])

                def vdma(eng, v_d, t):
                    eng.dma_start(out=t, in_=v_d[:].rearrange("(i p) -> p i", p=128))

                xm_sb = [
                    p1.tile([128, NKM], BF16, name="xm_sb", tag=f"xm{i}")
                    for i in range(NCH)
                ]

                def xmdma(eng, i, piece):
                    sl = slice(0, XMA) if piece == 0 else slice(XMA, NKM)
                    eng.dma_start(out=xm_sb[i][:, sl], in_=xm_d[bass.ts(i, 128), sl])

                # x rides ONLY the two HWDGE queues (SWDGE moves ~30 GB/s and
                # would gate the stats), split h0->SP / h1->ACT chunk-major so
                # chunk c's stats never wait on chunk c+1 traffic. All four
                # weights go to the slow SWDGE lane ordered by first use (wq
                # lands ~17us, wo ~40us -- each well before its consumer).
                # ACT lane: chain constants, x second-halves.
                nc.scalar.dma_start(out=ind2_sb, in_=ind2_d[:, :])
                nc.scalar.dma_start(out=ind_sb, in_=ind_d[:, :])
                vdma(nc.scalar, gam_d, gam_sb)
                vdma(nc.scalar, bet_d, bet_sb)
                xdma(nc.scalar, 0, 1)
                xdma(nc.scalar, 1, 1)
                xdma(nc.scalar, 2, 1)
                xdma(nc.scalar, 3, 1)
                # SP lane: small vectors, x first-halves, then all of xm.
                vdma(nc.sync, bq_d, bq_sb)
                vdma(nc.sync, bk_d, bk_sb)
                vdma(nc.sync, bo2_d, bo2_sb)
                nc.sync.dma_start(out=lm_sb, in_=lm_d[:].rearrange("(w p) -> p w", p=128))
                xdma(nc.sync, 0, 0)
                xdma(nc.sync, 1, 0)
                xdma(nc.sync, 2, 0)
                xdma(nc.sync, 3, 0)
                xmdma(nc.sync, 0, 0)
                xmdma(nc.sync, 1, 0)
                xmdma(nc.sync, 2, 0)
                xmdma(nc.sync, 3, 0)
                xmdma(nc.sync, 0, 1)
                xmdma(nc.sync, 1, 1)
                xmdma(nc.sync, 2, 1)
                xmdma(nc.sync, 3, 1)
                # gpsimd (SWDGE) lane: weights in order of first use.
                nc.gpsimd.dma_start(out=wq_all, in_=wq_d[:, :])
                nc.gpsimd.dma_start(out=wk_all, in_=wk_d[:, :])
                nc.gpsimd.dma_start(out=wv_all, in_=wv_d[:, :])
                nc.gpsimd.dma_start(out=wo_all, in_=wo_d[:, :])
                nc.vector.memset(onesp, 2.0)

                # Per-chunk stats: two accum_out passes on DVE (sum, sumsq)
                # replace bn_stats at ~2.3x the elem rate. Chain ops (tiny)
                # spread over ACT/DVE/PE.
                eps_sb = p1.tile([8, 1], F32, tag="eps")
                nc.vector.memset(eps_sb, EPS)
                scale4 = p1.tile([128, NCH], F32, tag="scale4")
                shift4 = p1.tile([128, NCH], F32, tag="shift4")
                scale_sb = [scale4[:, i : i + 1] for i in range(NCH)]
                shift_sb = [shift4[:, i : i + 1] for i in range(NCH)]
                rnorm = 1.0 / (GSIZE * HW)

                def stats_chunk(i):
                    xr = xf[i].rearrange("p (n f) -> p n f", f=512)
                    st = p1.tile([128, 8, 6], F32, name="bnst", tag="bnst", bufs=2)
                    for sg in range(8):
                        nc.vector.bn_stats(out=st[:, sg, :], in_=xr[:, sg, :])
                    st2 = p1.tile([128, 2], F32, name="st2", tag=f"st2_{i}")
                    stm = p1.tile([128, 2], F32, name="stm", tag=f"stm_{i}")
                    nc.vector.bn_aggr(out=stm, in_=st)
                    # convert (mean, var) to (sum, sumsq) for the shared chain:
                    # sum = mean*HW ; sumsq = (var+mean^2)*HW
                    nc.vector.tensor_scalar_mul(out=st2[:, 0:1], in0=stm[:, 0:1], scalar1=float(HW))
                    t_sq = p1.tile([128, 1], F32, name="t_sq", tag=f"t_sq{i}")
                    nc.vector.tensor_mul(out=t_sq, in0=stm[:, 0:1], in1=stm[:, 0:1])
                    nc.vector.scalar_tensor_tensor(
                        out=t_sq, in0=stm[:, 1:2], scalar=1.0, in1=t_sq,
                        op0=ALU.mult, op1=ALU.add,
                    )
                    nc.vector.tensor_scalar_mul(out=st2[:, 1:2], in0=t_sq, scalar1=float(HW))
                    return st2

                def chain_chunk(i, st2):
                    # st2 cols: sum, sumsq (per channel); group-reduce on PE
                    z_ps = p1p.tile([8, 2], F32, name="z_ps", tag="mr", bufs=2)
                    nc.tensor.matmul(z_ps, ind2_sb, st2, start=True, stop=True)
                    z_sb = p1.tile([8, 2], F32, name="z_sb", tag=f"z_sb{i}")
                    nc.scalar.copy(out=z_sb, in_=z_ps)
                    stat2 = p1.tile([8, 2], F32, name="stat2", tag=f"stat2_{i}")
                    nc.scalar.mul(out=stat2[:, 0:1], in_=z_sb[:, 0:1], mul=rnorm)
                    msq = p1.tile([8, 2], F32, name="msq", tag=f"msq{i}")
                    nc.scalar.activation(out=msq[:, 0:1], in_=stat2[:, 0:1], func=ACTF.Square)
                    # var = sumsq/N - mean^2 ; rstd = 1/sqrt(var+eps)
                    nc.vector.scalar_tensor_tensor(
                        out=msq[:, 1:2], in0=z_sb[:, 1:2], scalar=rnorm,
                        in1=msq[:, 0:1], op0=ALU.mult, op1=ALU.subtract,
                    )
                    nc.scalar.activation(out=msq[:, 1:2], in_=msq[:, 1:2], func=ACTF.Sqrt, bias=eps_sb)
                    nc.vector.reciprocal(out=stat2[:, 1:2], in_=msq[:, 1:2])
                    mr = p1p.tile([128, 2], F32, name="mr", tag="mr", bufs=2)
                    nc.tensor.matmul(mr, ind_sb, stat2, start=True, stop=True)
                    # scale = gamma * rstd ; shift = beta - mean * scale
                    nc.vector.tensor_mul(
                        out=scale_sb[i], in0=gam_sb[:, i : i + 1], in1=mr[:, 1:2]
                    )
                    tmp_sh = p1.tile([128, 1], F32, name="tmp_sh", tag=f"tmp_sh{i}")
                    nc.vector.tensor_scalar_mul(out=tmp_sh, in0=mr[:, 0:1], scalar1=scale_sb[i])
                    nc.vector.tensor_sub(out=shift_sb[i], in0=bet_sb[:, i : i + 1], in1=tmp_sh)

                st2s = {0: stats_chunk(0), 1: stats_chunk(1)}
                chain_chunk(0, st2s[0])
                st2s[2] = stats_chunk(2)
                chain_chunk(1, st2s[1])
                st2s[3] = stats_chunk(3)
                chain_chunk(2, st2s[2])
                chain_chunk(3, st2s[3])

                # projections. q over the local query half (from xf);
                # k/v only over the mask-compacted key columns (xm).
                # hn/hm produced in fp8 pair layout [128, 2, 512] on ACT
                # (per-partition scale+bias activation); q/k bias+1/16 on DVE.
                for nw in range(NW // 2):
                    nsl = bass.ts(nw, 512)
                    hn = []
                    for cp in range(NCP):
                        h_t = p1.tile([128, 2, 512], F8, name="hn", tag="hn", bufs=8)
                        for j in range(2):
                            c = 2 * cp + j
                            nc.scalar.activation(
                                out=h_t[:, j, :], in_=xf[c][:, nsl],
                                func=ACTF.Identity,
                                scale=scale_sb[c], bias=shift_sb[c],
                            )
                        hn.append(h_t)
                    for co in range(NCH):
                        pq = p1p.tile([128, 512], F32, name="pq", tag="pq", bufs=2)
                        for cp in range(NCP):
                            nc.tensor.matmul(
                                pq, wqp[cp][:, :, bass.ts(co, 128)], hn[cp],
                                start=(cp == 0), stop=(cp == NCP - 1),
                                perf_mode=PERF,
                            )
                        nc.vector.tensor_scalar(
                            out=qp[co // 2][:, co % 2, nsl], in0=pq,
                            scalar1=1.0 / WSCL, scalar2=bq_sb[:, co : co + 1],
                            op0=ALU.mult, op1=ALU.add,
                        )
                for mw in range((NKM + 511) // 512):
                    lo = mw * 512
                    wsz = min(512, NKM - lo)
                    msl = slice(lo, lo + wsz)
                    hm = []
                    for cp in range(NCP):
                        h_t = p1.tile([128, 2, 512], F8, name="hm", tag="hn", bufs=8)
                        for j in range(2):
                            c = 2 * cp + j
                            nc.scalar.activation(
                                out=h_t[:, j, :wsz], in_=xm_sb[c][:, msl],
                                func=ACTF.Identity,
                                scale=scale_sb[c], bias=shift_sb[c],
                            )
                        hm.append(h_t)
                    for co in range(NCH):
                        pk = p1p.tile([128, 512], F32, name="pk", tag="pk", bufs=2)
                        for cp in range(NCP):
                            nc.tensor.matmul(
                                pk[:, :wsz], wkp[cp][:, :, bass.ts(co, 128)],
                                hm[cp][:, :, :wsz],
                                start=(cp == 0), stop=(cp == NCP - 1),
                                perf_mode=PERF,
                            )
                        nc.vector.tensor_scalar(
                            out=kp[co // 2][:, co % 2, msl], in0=pk[:, :wsz],
                            scalar1=1.0 / WSCL, scalar2=bk_sb[:, co : co + 1],
                            op0=ALU.mult, op1=ALU.add,
                        )
                    # v, produced transposed and x16: vt[key, c_out] = hm^T @ (16 Wv)
                    for kw in range(wsz // 128):
                        pv = p1p.tile([128, C], F32, name="pv", tag="pv", bufs=2)
                        for cp in range(NCP):
                            nc.tensor.matmul(
                                pv, hm[cp][:, :, bass.ts(kw, 128)], wvp[cp],
                                start=(cp == 0), stop=(cp == NCP - 1),
                                perf_mode=PERF,
                            )
                        kwg = mw * 4 + kw
                        nc.vector.tensor_copy(out=vtp[kwg // 2][:, kwg % 2, :], in_=pv)

            # ================= phase 2: attention =================
            with (
                tc.tile_pool(name="ph2", bufs=1) as p2,
                tc.tile_pool(name="ph2psum", bufs=1, space="PSUM") as p2p,
            ):
                def emit_scores(qt, wp):
                    """scores + exp for window pair wp against query tile qt;
                    returns the fp8 probability pair tile."""
                    qsl = bass.ts(qt, QT)
                    ptp = p2.tile([128, 2, QT], F8, name="ptp", tag="pt", bufs=3)
                    for j in range(2):
                        w = 2 * wp + j
                        sc = p2p.tile([128, QT], F32, name="sc", tag="sc", bufs=3)
                        for cp in range(NCP):
                            nc.tensor.matmul(
                                sc, kp[cp][:, :, bass.ts(w, 128)],
                                qp[cp][:, :, qsl],
                                start=(cp == 0), stop=(cp == NCP - 1),
                                perf_mode=PERF,
                            )
                        # p = exp(s/sqrt(C) + logmask_k - log4)
                        nc.scalar.activation(
                            out=ptp[:, j, :], in_=sc, func=ACTF.Exp,
                            bias=lm_sb[:, w : w + 1], scale=qscale,
                        )
                    return ptp

                items = [(qt, wp) for qt in range(NQT) for wp in range(NKWP)]
                ptp_next = emit_scores(*items[0])
                out_ps = None
                for idx, (qt, wp) in enumerate(items):
                    qsl = bass.ts(qt, QT)
                    ptp_cur = ptp_next
                    # prefetch the next pair's scores so the PE never waits
                    # on this pair's Exp
                    if idx + 1 < len(items):
                        ptp_next = emit_scores(*items[idx + 1])
                    if wp == 0:
                        out_ps = [
                            p2p.tile([128, QT], F32, name="out_ps", tag="out", bufs=4)
                            for _ in range(NCH)
                        ]
                        ds_ps = p2p.tile([128, QT], F32, name="ds_ps", tag="ds", bufs=1)
                    for c in range(NCH):
                        nc.tensor.matmul(
                            out_ps[c], vtp[wp][:, :, bass.ts(c, 128)], ptp_cur,
                            start=(wp == 0), stop=(wp == NKWP - 1),
                            perf_mode=PERF,
                        )
                    nc.tensor.matmul(
                        ds_ps, onesp, ptp_cur,
                        start=(wp == 0), stop=(wp == NKWP - 1),
                        perf_mode=PERF,
                    )
                    if wp != NKWP - 1:
                        continue
                    # ---- query-tile tail ----
                    # out_ps = 16*sum(p v), ds_ps = 2*sum(p). Cast the
                    # unnormalized context to fp8 as sum(p v)/8 (|.| < 90);
                    # the per-query softmax denominator is applied after Wo.
                    onp = [
                        p2.tile([128, 2, QT], F8, name="onp", tag="onp", bufs=2)
                        for _ in range(NCP)
                    ]
                    for c in range(NCH):
                        nc.vector.tensor_scalar_mul(
                            out=onp[c // 2][:, c % 2, :], in0=out_ps[c],
                            scalar1=1.0 / (WSCL * 8.0),
                        )
                    # dinv = 1/(2 sum p); off the PE critical path
                    dinv = p2.tile([128, QT], F32, name="dinv", tag="dinv", bufs=2)
                    nc.vector.reciprocal(out=dinv, in_=ds_ps)
                    for co in range(NCH):
                        pj = p2p.tile([128, QT], F32, name="pj", tag="out", bufs=4)
                        for cp in range(NCP):
                            nc.tensor.matmul(
                                pj, wop[cp][:, :, bass.ts(co, 128)], onp[cp],
                                start=(cp == 0), stop=(cp == NCP - 1),
                                perf_mode=PERF,
                            )
                        # pj = 2*(Wo sum(p v)); pj*dinv = Wo out exactly;
                        # y = pj*dinv + bo2 + x in two DVE ops
                        t2 = p2.tile([128, QT], F32, name="t2", tag="t2", bufs=3)
                        nc.vector.tensor_mul(out=t2, in0=pj, in1=dinv)
                        y_t = p2.tile([128, QT], F32, name="y_t", tag="yt", bufs=3)
                        nc.vector.scalar_tensor_tensor(
                            out=y_t, in0=t2, scalar=bo2_sb[:, co : co + 1],
                            in1=xf[co][:, qsl], op0=ALU.add, op1=ALU.add,
                        )
                        nc.sync.dma_start(out=y_d[bass.ts(co, 128), qsl], in_=y_t)

            loop_ctx.close()

    nc.finalize()
    return nc


_prog_cache = {}


def _get_program(loop_n: int = 1):
    if loop_n not in _prog_cache:
        _prog_cache[loop_n] = build_program(loop_n)
    return _prog_cache[loop_n]


def _to_f8(a):
    return np.clip(a, -240.0, 240.0).astype(ml_dtypes.float8_e4m3)


def _prearrange_w(W):
    # [p, cp*1024 + j*512 + c] = 16*W.T[cp*256 + j*128 + p, c]
    arr = np.ascontiguousarray(np.asarray(W, np.float32).T) * WSCL
    pre = arr.reshape(2, 2, 128, C).transpose(2, 0, 1, 3).reshape(128, 4 * C)
    return _to_f8(pre)


def _prep_in_maps(x, mask, gamma, beta, Wq, bq, Wk, bk, Wv, bv, Wo, bo):
    x = np.asarray(x, np.float32).reshape(B, C, HW)
    mask = np.asarray(mask, np.float32)
    bf = ml_dtypes.bfloat16
    shared = {
        "gamma": np.asarray(gamma, np.float32),
        "beta": np.asarray(beta, np.float32),
        "wqt": _prearrange_w(Wq),
        "wkt": _prearrange_w(Wk),
        "wvt": _prearrange_w(Wv),
        "wot": _prearrange_w(Wo),
        "bq2": np.asarray(bq, np.float32),
        "bk": np.asarray(bk, np.float32),
        "bo2": (np.asarray(Wo, np.float32) @ np.asarray(bv, np.float32)
                + np.asarray(bo, np.float32)).astype(np.float32),
        "ind": (np.arange(128)[None, :] // GSIZE == np.arange(8)[:, None]).astype(
            np.float32
        ),
        "ind2": (np.arange(128)[:, None] // GSIZE == np.arange(8)[None, :]).astype(
            np.float32
        ),
    }
    in_maps = []
    for core in range(8):
        b, half = core // 2, core % 2
        xb, mb = x[b], mask[b]
        if half == 1:
            xb = np.concatenate([xb[:, NQ:], xb[:, :NQ]], axis=1)
            mb = np.concatenate([mb[NQ:], mb[:NQ]])
        # compact the keys: only masked-in columns take part in attention
        idx = np.nonzero(mb > 0.5)[0]
        nk = len(idx)
        assert nk <= NKM, f"mask density too high: {nk} > {NKM}"
        xm = np.zeros((C, NKM), dtype=bf)
        xm[:, :nk] = xb[:, idx].astype(bf)
        lm = np.full(NKM, NEG, np.float32)
        lm[:nk] = LOGSHIFT
        in_maps.append(
            {"xbf": xb.astype(bf), "xm": xm, "lmask": lm, **shared}
        )
    return in_maps


def kernel(x, mask, gamma, beta, Wq, bq, Wk, bk, Wv, bv, Wo, bo):
    nc = _get_program()
    in_maps = _prep_in_maps(x, mask, gamma, beta, Wq, bq, Wk, bk, Wv, bv, Wo, bo)
    res = run_bass_kernel_spmd(nc, in_maps, list(range(8)))
    out = np.empty((B, C, HW), np.float32)
    for core in range(8):
        b, half = core // 2, core % 2
        out[b, :, half * NQ : (half + 1) * NQ] = res.results[core]["y"]
    return out.reshape(B, C, HGT, WID)
